# revision 34
# baseline (speedup 1.0000x reference)
"""Trainium2 Bass kernel for nn_Att_mlp_softmax (GNN message passing).

Reference computation:
    e = relu(h @ W1 + b1) @ W2 + b2                       # [N, 1] per-node score
    att = softmax(where(G > 0, e.T broadcast, -9e15))     # row-wise over neighbors
    out = (G.sum(-1))[:, None] * (att @ h)                # degree-rescaled aggregation

The pre-softmax score of entry (i, j) depends only on column j, so the masked
softmax collapses algebraically: with w = exp(e - c) and H' = [w * h | w | 1]
(N x 130):
    Y = G @ H'
    out = Y[:, 129] * Y[:, :128] / Y[:, 128]
Any constant shift c (and b2) cancels exactly in the ratio, so exp needs no
bias at all; fp16 range is ample for this data.

Precision/perf (correctness gate is rel_err < 2e-2, so single 16-bit precision
is ample): G is an exact 0/1 mask streamed in fp8e4 (1 byte/elem HBM traffic,
FWL 4x weight loads); H' is fp16 (upcast to e10m11 inside the PE). PSUM
accumulates fp32. Measured end-to-end rel err ~1e-3.

Distribution: G row-sharded across 8 NeuronCores (1024 rows each); h and MLP
weights replicated; no collectives. Each core's shard is passed pre-transposed
as gTr[p, a, i] = G[base+i, a*128+p] so stationary tiles are contiguous.

Score MLP: z = h @ W1 is computed j-on-partitions (stationary hT tile,
moving W1), then e = sum_k relu(z)*W2 via a fused DVE max+mult (stt) and a
free-dim reduce, and w = exp(e) on the Scalar engine -- shift-free, since any
constant in e cancels in the num/den ratio. Group 0 runs this per-chunk (low
latency, H' builds on Scalar right behind each exp) so the main loop starts
as soon as the first G half-group lands; later groups run batched. When b1
is nonzero a bias-add is prepended (general fallback).

Pipeline (per 8-chunk group g):
    PE:  z MMs for group g+1 interleave with the 64 main G matmuls of group g
         (stationary G tile fp8 + FWL, moving H' 130 cols at the 57 ns/MM
         issue floor, 8 accumulators packed 3 per PSUM bank -- only the first
         slice per bank may issue start=True since the has_written clear is
         bank-wide). Group 0 is split in half-groups to start earlier.
    DMA: ALL heavy traffic on the sync queue (the fastest pipe; more queues
         or finer transfers measured slower) in exact need-order
         [hT0, hc0, G0a, G0b, hT1, G1, hc1, ...]; smalls on gpsimd; DMA
         triggers cost ~600 ns of issuing-engine time so compute engines
         never host mid-stream triggers; act tables preloaded at t=0.
The last group runs bank-major (0,3,6,1,4,7,2,5) so each PSUM bank's epilogue
(deg/den rescale) overlaps the remaining banks' matmuls without WAR stalls,
with per-bank output DMAs.
"""

import numpy as np

N = 8192
D = 128
HID = 64
N_CORES = 8
ROWS = N // N_CORES          # 1024 output rows per core
JC = N // 128                # 64 contraction chunks of 128
GRP = 8                      # chunks per group (1 MB fp8 G DMA each)
NG = JC // GRP               # 8 groups
NCOL = 130                   # moving columns: [w*h (128) | w | 1]
ESHIFT = -4.0                # general-b1 path only: keeps w in range

_cache = {}


def _install_axon_hooks_shim():
    """Provide antenv.axon_hooks if the image lacks it (trn_boot step 6).

    concourse.bass_utils imports it unconditionally when BASS_TRACE is set;
    without the shim that import crashes instead of degrading.
    """
    import contextlib
    import ctypes
    import sys
    import types

    try:
        import antenv.axon_hooks  # noqa: F401
        return
    except ImportError:
        pass

    so_path = "/opt/axon/libaxon_pjrt.so"

    def _make_hook():
        try:
            lib = ctypes.CDLL(so_path)
        except OSError:
            return None
        if not hasattr(lib, "axon_start_nrt_profile"):
            return None
        lib.axon_start_nrt_profile.argtypes = [
            ctypes.POINTER(ctypes.c_int64),
            ctypes.c_size_t,
        ]
        lib.axon_start_nrt_profile.restype = ctypes.c_int64
        lib.axon_stop_nrt_profile.argtypes = [ctypes.c_char_p]
        lib.axon_stop_nrt_profile.restype = ctypes.c_int64

        @contextlib.contextmanager
        def _hook(output_dir, device_ids):
            import jax

            jax.devices()
            if device_ids:
                ids = (ctypes.c_int64 * len(device_ids))(*device_ids)
                rc = lib.axon_start_nrt_profile(ids, len(device_ids))
            else:
                rc = lib.axon_start_nrt_profile(None, 0)
            if rc != 0:
                raise RuntimeError(f"axon_start_nrt_profile rc={rc}")
            try:
                yield
            finally:
                lib.axon_stop_nrt_profile(str(output_dir).encode())

        return _hook

    mod = types.ModuleType("antenv.axon_hooks")
    _holder = {"hook": _make_hook()}
    mod.set_axon_ntff_profile_hook = lambda h: _holder.__setitem__("hook", h)
    mod.get_axon_ntff_profile_hook = lambda: _holder["hook"]
    sys.modules["antenv.axon_hooks"] = mod
    try:
        import antenv

        antenv.axon_hooks = mod
    except ImportError:
        pass


def build_nc(b1_zero, enable_asserts=False):
    """Build + compile the per-core Bass program (identical on all 8 cores).

    b1_zero: when True the score chain skips the bias add (2 DVE ops per
        group); otherwise the general batched DVE chain adds b1 first.
    """
    from concourse import bacc, mybir, tile

    f32 = mybir.dt.float32
    fp16 = mybir.dt.float16
    fp8 = mybir.dt.float8e4
    AF = mybir.ActivationFunctionType
    ALU = mybir.AluOpType
    AX = mybir.AxisListType

    nc = bacc.Bacc(
        "TRN2",
        target_bir_lowering=False,
        debug=False,
        enable_asserts=enable_asserts,
        num_devices=N_CORES,
    )
    gTr = nc.dram_tensor("gTr", [128, JC, ROWS], fp8, kind="ExternalInput").ap()
    hT = nc.dram_tensor("hT", [D, N], fp16, kind="ExternalInput").ap()
    hc = nc.dram_tensor("hc", [128, JC, D + 1], fp16, kind="ExternalInput").ap()
    W1s = nc.dram_tensor("W1s", [D, HID], fp16, kind="ExternalInput").ap()
    w2bt = nc.dram_tensor("w2bt", [128, GRP, HID], fp16, kind="ExternalInput").ap()
    if not b1_zero:
        b1bt = nc.dram_tensor(
            "b1bt", [128, GRP, HID], fp16, kind="ExternalInput"
        ).ap()
        ebias = nc.dram_tensor("ebias", [128, 1], f32, kind="ExternalInput").ap()
    out = nc.dram_tensor("out", [128, 8, D], fp16, kind="ExternalOutput").ap()

    with tile.TileContext(nc) as tc:
        with (
            tc.tile_pool(name="const", bufs=1) as cpool,
            tc.tile_pool(name="gbuf", bufs=8) as gpool,
            tc.tile_pool(name="hpbuf", bufs=16) as hpool,
            tc.tile_pool(name="sbuf", bufs=2) as spool,
            tc.tile_pool(name="outbuf", bufs=1) as opool,
            tc.tile_pool(name="ps_z", bufs=2, space="PSUM") as zpool,
            tc.tile_pool(name="ps_w", bufs=1, space="PSUM") as wpool,
            tc.tile_pool(name="ps_acc", bufs=3, space="PSUM") as ps_acc,
        ):
            # ---- smalls on the gpsimd queue ----
            W1s_sb = cpool.tile([D, HID], fp16)
            nc.gpsimd.dma_start(W1s_sb[:], W1s[:])
            w2bt_sb = cpool.tile([128, GRP, HID], fp16)
            nc.gpsimd.dma_start(w2bt_sb[:], w2bt[:])
            if not b1_zero:
                b1bt_sb = cpool.tile([128, GRP, HID], fp16)
                nc.gpsimd.dma_start(b1bt_sb[:], b1bt[:])
                ebias_sb = cpool.tile([128, 1], f32)
                nc.gpsimd.dma_start(ebias_sb[:], ebias[:])

            # ---- heavy traffic: ONE queue (sync), exact need-order ----
            hT_sb = cpool.tile([D, N], fp16)
            hc_sb = cpool.tile([128, JC, D + 1], fp16)
            gts = {}

            def dma_group(g):
                sl = slice(g * GRP, (g + 1) * GRP)
                hsl = slice(g * (N // 8), (g + 1) * (N // 8))
                nc.sync.dma_start(hT_sb[:, hsl], hT[:, hsl])
                gt = gpool.tile([128, GRP, ROWS], fp8, tag="gt", name=f"gt{g}")
                nc.sync.dma_start(gt[:], gTr[:, sl, :])
                nc.sync.dma_start(hc_sb[:, sl, :], hc[:, sl, :])
                gts[g] = gt

            # group 0 split in half-groups at every level (DMA, z MMs, main
            # MMs) so the first 32 main matmuls start on ~0.7 MB of data.
            # All heavy traffic stays on sync -- the fastest queue.
            nc.sync.dma_start(hT_sb[:, 0:1024], hT[:, 0:1024])
            nc.sync.dma_start(hc_sb[:, 0:GRP, :], hc[:, 0:GRP, :])
            g0ts = []
            for h in range(2):
                qsl = slice(h * 4, (h + 1) * 4)
                g0t = gpool.tile([128, 4, ROWS], fp8, tag="g0t", name=f"g0t{h}",
                                 bufs=2)
                nc.sync.dma_start(g0t[:], gTr[:, qsl, :])
                g0ts.append(g0t)
            for g in range(1, NG):
                dma_group(g)

            # preset the constant ones column (col 129) in every hp ring
            # buffer once; later ts_mul builds write only cols 0:129
            for b in range(16):
                t = hpool.tile([128, NCOL], fp16, tag="hp", name=f"hpinit{b}")
                nc.vector.memset(t[:, 129:130], 1.0)

            # wones[:, 0, jc] = w_jc (written by ACT exp). Writes stay
            # contiguous in the innermost dim (non-contiguous engine writes
            # mis-lower on HW); reads may stride.
            wones = cpool.tile([128, 2, JC], f32)
            zero1 = cpool.tile([128, 1], f32)
            nc.vector.memset(zero1[:], 0.0)

            # ---- PE warmup (HAM un-throttle) + activation-table preloads,
            # all dependency-free so they run during the first DMAs ----
            warm = cpool.tile([128, 128], fp16)
            nc.vector.memset(warm[:], 0.0)
            junk0 = spool.tile([128, 1], f32, tag="junk0", bufs=1)
            nc.scalar.activation(junk0[:], zero1[:], AF.Exp)
            nc.scalar.activation(junk0[:], zero1[:], AF.Copy)
            pwarm = wpool.tile([128, 128], f32, tag="pwarm")
            for _ in range(22):
                nc.tensor.matmul(pwarm[:], warm[:], warm[:], start=True, stop=True)

            # pack 3 accumulators per PSUM bank (3 * 130 f32 = 1560 B of 2 KB)
            acctiles = [
                ps_acc.tile([128, 3, NCOL], f32, tag="acc", name=f"accb{i}")
                for i in range(3)
            ]
            accs = [acctiles[i // 3][:, i % 3, :] for i in range(8)]

            def emit_z0_half(hh):
                """Group-0 half-block: 4 z MMs + per-chunk chains, hp on ACT."""
                zps = zpool.tile([128, 4, HID], f32, tag="z0", name=f"z0h{hh}",
                                 bufs=2)
                for k4 in range(4):
                    c = hh * 4 + k4
                    nc.tensor.matmul(
                        zps[:, k4, :],
                        hT_sb[:, c * 128 : (c + 1) * 128],
                        W1s_sb[:],
                        start=True,
                        stop=True,
                    )
                for k4 in range(4):
                    k = hh * 4 + k4
                    prodk = spool.tile([128, HID], fp16, tag="prodk",
                                       name=f"prodk{k}", bufs=3)
                    nc.vector.scalar_tensor_tensor(
                        prodk[:], zps[:, k4, :], 0.0, w2bt_sb[:, k % GRP, :],
                        op0=ALU.max, op1=ALU.mult,
                    )
                    e1 = spool.tile([128, 1], f32, tag="e1",
                                    name=f"e1{k}", bufs=3)
                    nc.vector.tensor_reduce(
                        e1[:], prodk[:], axis=AX.X, op=ALU.add
                    )
                    # shift-free: any constant in e cancels in the ratio
                    nc.scalar.activation(
                        wones[:, 0, k : k + 1], e1[:], AF.Exp
                    )
                    hps0.append(build_hp(k, on_act=True))

            def emit_z(g):
                """MLP z for the 8 chunks of group g: z[j, k] on j-partitions,
                then a batched score chain -> w."""
                zps = zpool.tile([128, GRP, HID], f32, tag="z", name=f"z{g}")
                for k in range(GRP):
                    c = g * GRP + k
                    nc.tensor.matmul(
                        zps[:, k, :],
                        hT_sb[:, c * 128 : (c + 1) * 128],
                        W1s_sb[:],
                        start=True,
                        stop=True,
                    )
                if b1_zero:
                    prod = spool.tile([128, GRP, HID], fp16, tag="prod")
                    nc.vector.scalar_tensor_tensor(
                        prod[:], zps[:], 0.0, w2bt_sb[:], op0=ALU.max,
                        op1=ALU.mult,
                    )
                    e8 = spool.tile([128, GRP], f32, tag="e8")
                    nc.vector.tensor_reduce(e8[:], prod[:], axis=AX.X, op=ALU.add)
                    nc.scalar.activation(
                        wones[:, 0, g * GRP : (g + 1) * GRP], e8[:], AF.Exp
                    )
                else:
                    # general-b1 fallback: batched DVE relu-dot chain
                    zb = spool.tile([128, GRP, HID], fp16, tag="zb")
                    nc.vector.tensor_tensor(zb[:], zps[:], b1bt_sb[:], op=ALU.add)
                    prod = spool.tile([128, GRP, HID], fp16, tag="prod")
                    nc.vector.scalar_tensor_tensor(
                        prod[:], zb[:], 0.0, w2bt_sb[:], op0=ALU.max, op1=ALU.mult
                    )
                    e8 = spool.tile([128, GRP], f32, tag="e8")
                    nc.vector.tensor_reduce(e8[:], prod[:], axis=AX.X, op=ALU.add)
                    nc.scalar.activation(
                        wones[:, 0, g * GRP : (g + 1) * GRP], e8[:], AF.Exp,
                        bias=ebias_sb[:],
                    )

            def build_hp(jc, on_act=False):
                """Just-in-time H' chunk: [w*h | w | 1] fp16.

                hc carries a host-side ones column, so ONE scaled copy makes
                [w*h | w]; the constant 1 in col 129 was preset per buffer.
                Builds alternate between Vector (ts_mul) and Scalar (Copy
                with per-partition scale) to balance engine load.
                """
                hp = hpool.tile([128, NCOL], fp16, tag="hp", name=f"hp{jc}")
                if on_act:
                    nc.scalar.activation(
                        hp[:, 0:129], hc_sb[:, jc, :], AF.Copy,
                        scale=wones[:, 0, jc : jc + 1],
                    )
                else:
                    nc.vector.tensor_scalar_mul(
                        hp[:, 0:129], hc_sb[:, jc, :], wones[:, 0, jc : jc + 1]
                    )
                return hp

            def epilogue(it):
                """out rows of bank it: deg/den rescale into ot_all."""
                den = spool.tile([128, 1], f32, tag="den", name=f"den{it}", bufs=8)
                nc.vector.tensor_scalar_add(den[:], accs[it][:, 128:129], 1e-30)
                rc = spool.tile([128, 1], f32, tag="rc", name=f"rc{it}", bufs=8)
                nc.vector.reciprocal(rc[:], den[:])
                r = spool.tile([128, 1], f32, tag="r", name=f"r{it}", bufs=8)
                nc.vector.tensor_tensor(r[:], rc[:], accs[it][:, 129:130], op=ALU.mult)
                nc.vector.tensor_scalar_mul(
                    ot_all[:, it, :], accs[it][:, 0:128], r[:]
                )

            ot_all = opool.tile([128, 8, D], fp16, tag="ot_all", bufs=1)

            hps0 = []
            # group 0: half-blocks interleaved z -> main
            for hh in range(2):
                emit_z0_half(hh)
                for k4 in range(4):
                    jc = hh * 4 + k4
                    hp = hps0[jc]
                    for it in range(8):
                        # start=True clears has_written for the WHOLE psum
                        # bank, so only the first slice sharing each bank
                        # may issue it; siblings then init via overwrite
                        # (has_written=0) on their first matmul.
                        nc.tensor.matmul(
                            accs[it][:],
                            g0ts[hh][:, k4, it * 128 : (it + 1) * 128],
                            hp[:],
                            start=(jc == 0 and it % 3 == 0),
                            stop=False,
                        )
            for g in range(1, NG):
                if g == 1:
                    emit_z(1)
                    emit_z(2)
                elif g + 1 < NG:
                    emit_z(g + 1)
                gt = gts.pop(g)

                if g < NG - 1:
                    for k in range(GRP):
                        jc = g * GRP + k
                        hp = build_hp(jc, on_act=(k % 2 == 1))
                        for it in range(8):
                            nc.tensor.matmul(
                                accs[it][:],
                                gt[:, k, it * 128 : (it + 1) * 128],
                                hp[:],
                                start=False,
                                stop=False,
                            )
                else:
                    # last group bank-major: each bank's epilogue overlaps the
                    # remaining banks' matmuls. Visit order hops across PSUM
                    # banks (accs share banks in triples) so an epilogue's
                    # DVE reads never WAR-block the next bank's matmuls.
                    hps = [build_hp(g * GRP + k, on_act=(k % 2 == 1))
                           for k in range(GRP)]
                    done = [0, 0, 0]
                    for it in (0, 3, 6, 1, 4, 7, 2, 5):
                        for k in range(GRP):
                            nc.tensor.matmul(
                                accs[it][:],
                                gt[:, k, it * 128 : (it + 1) * 128],
                                hps[k][:],
                                start=False,
                                stop=(k == GRP - 1),
                            )
                        epilogue(it)
                        b = it // 3
                        done[b] += 1
                        if done[b] == (2 if b == 2 else 3):
                            sl = slice(3 * b, min(3 * b + 3, 8))
                            nc.scalar.dma_start(out[:, sl, :], ot_all[:, sl, :])

    nc.compile()
    return nc


def make_in_maps(graph_info, h, W1, b1, W2, b2):
    """Shard + lay out the full inputs for the 8 cores."""
    import ml_dtypes

    fp16 = np.float16
    fp8 = ml_dtypes.float8_e4m3

    W1 = np.asarray(W1, np.float32)
    W2 = np.asarray(W2, np.float32).reshape(HID)
    b1 = np.asarray(b1, np.float32)
    b1_zero = not np.any(b1)

    h = np.asarray(h, np.float32)
    hT = np.ascontiguousarray(h.T).astype(fp16)                # [D, N]
    hcm = np.ascontiguousarray(
        np.concatenate(
            [h.reshape(JC, 128, D), np.ones((JC, 128, 1), np.float32)], axis=2
        ).transpose(1, 0, 2)                                   # [128, JC, D+1]
    ).astype(fp16)

    base = {
        "hT": hT,
        "hc": hcm,
        "W1s": W1.astype(fp16),
        "w2bt": np.ascontiguousarray(
            np.broadcast_to(W2, (128, GRP, HID))
        ).astype(fp16),
    }
    if not b1_zero:
        base["b1bt"] = np.ascontiguousarray(
            np.broadcast_to(b1, (128, GRP, HID))
        ).astype(fp16)
        base["ebias"] = np.full(
            (128, 1), float(np.asarray(b2).reshape(-1)[0]) + ESHIFT, np.float32
        )

    g8 = np.asarray(graph_info, np.float32).astype(fp8)        # exact 0/1
    in_maps = []
    for c in range(N_CORES):
        shard = g8[c * ROWS : (c + 1) * ROWS]                  # [1024, N]
        gTr = np.ascontiguousarray(
            shard.reshape(ROWS, JC, 128).transpose(2, 1, 0)    # [128, JC, 1024]
        )
        in_maps.append(dict(base, gTr=gTr))
    return in_maps


def kernel(graph_info, h, W1, b1, W2, b2):
    _install_axon_hooks_shim()
    from concourse.bass_utils import run_bass_kernel_spmd

    b1_zero = not np.any(np.asarray(b1))
    key = ("nc", b1_zero)
    if key not in _cache:
        _cache[key] = build_nc(b1_zero)
        _cache["nc"] = _cache[key]
    nc = _cache[key]

    in_maps = make_in_maps(graph_info, h, W1, b1, W2, b2)
    res = run_bass_kernel_spmd(nc, in_maps, list(range(N_CORES)))
    return np.concatenate(
        [
            res.results[c]["out"].transpose(1, 0, 2).reshape(ROWS, D)
            for c in range(N_CORES)
        ],
        axis=0,
    ).astype(np.float32)


# revision 35
# speedup vs baseline: 1.0348x; 1.0348x over previous
"""Trainium2 Bass kernel for nn_Att_mlp_softmax (GNN message passing).

Reference computation:
    e = relu(h @ W1 + b1) @ W2 + b2                       # [N, 1] per-node score
    att = softmax(where(G > 0, e.T broadcast, -9e15))     # row-wise over neighbors
    out = (G.sum(-1))[:, None] * (att @ h)                # degree-rescaled aggregation

The pre-softmax score of entry (i, j) depends only on column j, so the masked
softmax collapses algebraically: with w = exp(e - c) and H' = [w * h | w | 1]
(N x 130):
    Y = G @ H'
    out = Y[:, 129] * Y[:, :128] / Y[:, 128]
Any constant shift c (and b2) cancels exactly in the ratio, so exp needs no
bias at all; fp16 range is ample for this data.

Precision/perf (correctness gate is rel_err < 2e-2, so single 16-bit precision
is ample): G is an exact 0/1 mask streamed in fp8e4 (1 byte/elem HBM traffic,
FWL 4x weight loads); H' is fp16 (upcast to e10m11 inside the PE). PSUM
accumulates fp32. Measured end-to-end rel err ~1e-3.

Distribution: G row-sharded across 8 NeuronCores (1024 rows each); h and MLP
weights replicated; no collectives. Each core's shard is passed pre-transposed
as gTr[p, a, i] = G[base+i, a*128+p] so stationary tiles are contiguous.

Score MLP: z = h @ W1 is computed j-on-partitions (stationary hT tile,
moving W1), then e = sum_k relu(z)*W2 via a fused DVE max+mult (stt) and a
free-dim reduce, and w = exp(e) on the Scalar engine -- shift-free, since any
constant in e cancels in the num/den ratio. Group 0 runs this per-chunk (low
latency, H' builds on Scalar right behind each exp) so the main loop starts
as soon as the first G half-group lands; later groups run batched. When b1
is nonzero a bias-add is prepended (general fallback).

Pipeline (per 8-chunk group g):
    PE:  z MMs for group g+1 interleave with the 64 main G matmuls of group g
         (stationary G tile fp8 + FWL, moving H' 130 cols at the 57 ns/MM
         issue floor, 8 accumulators packed 3 per PSUM bank -- only the first
         slice per bank may issue start=True since the has_written clear is
         bank-wide). Group 0 is split in half-groups to start earlier.
    DMA: ALL heavy traffic on the sync queue (the fastest pipe; more queues
         or finer transfers measured slower) in exact need-order
         [hT0, hc0, G0a, G0b, hT1, G1, hc1, ...]; smalls on gpsimd; DMA
         triggers cost ~600 ns of issuing-engine time so compute engines
         never host mid-stream triggers; act tables preloaded at t=0.
The last group runs bank-major (0,3,6,1,4,7,2,5) so each PSUM bank's epilogue
(deg/den rescale) overlaps the remaining banks' matmuls without WAR stalls,
with per-bank output DMAs.
"""

import numpy as np

N = 8192
D = 128
HID = 64
N_CORES = 8
ROWS = N // N_CORES          # 1024 output rows per core
JC = N // 128                # 64 contraction chunks of 128
GRP = 8                      # chunks per group (1 MB fp8 G DMA each)
NG = JC // GRP               # 8 groups
NCOL = 130                   # moving columns: [w*h (128) | w | 1]
ESHIFT = -4.0                # general-b1 path only: keeps w in range

_cache = {}


def _install_axon_hooks_shim():
    """Provide antenv.axon_hooks if the image lacks it (trn_boot step 6).

    concourse.bass_utils imports it unconditionally when BASS_TRACE is set;
    without the shim that import crashes instead of degrading.
    """
    import contextlib
    import ctypes
    import sys
    import types

    try:
        import antenv.axon_hooks  # noqa: F401
        return
    except ImportError:
        pass

    so_path = "/opt/axon/libaxon_pjrt.so"

    def _make_hook():
        try:
            lib = ctypes.CDLL(so_path)
        except OSError:
            return None
        if not hasattr(lib, "axon_start_nrt_profile"):
            return None
        lib.axon_start_nrt_profile.argtypes = [
            ctypes.POINTER(ctypes.c_int64),
            ctypes.c_size_t,
        ]
        lib.axon_start_nrt_profile.restype = ctypes.c_int64
        lib.axon_stop_nrt_profile.argtypes = [ctypes.c_char_p]
        lib.axon_stop_nrt_profile.restype = ctypes.c_int64

        @contextlib.contextmanager
        def _hook(output_dir, device_ids):
            import jax

            jax.devices()
            if device_ids:
                ids = (ctypes.c_int64 * len(device_ids))(*device_ids)
                rc = lib.axon_start_nrt_profile(ids, len(device_ids))
            else:
                rc = lib.axon_start_nrt_profile(None, 0)
            if rc != 0:
                raise RuntimeError(f"axon_start_nrt_profile rc={rc}")
            try:
                yield
            finally:
                lib.axon_stop_nrt_profile(str(output_dir).encode())

        return _hook

    mod = types.ModuleType("antenv.axon_hooks")
    _holder = {"hook": _make_hook()}
    mod.set_axon_ntff_profile_hook = lambda h: _holder.__setitem__("hook", h)
    mod.get_axon_ntff_profile_hook = lambda: _holder["hook"]
    sys.modules["antenv.axon_hooks"] = mod
    try:
        import antenv

        antenv.axon_hooks = mod
    except ImportError:
        pass


def build_nc(b1_zero, enable_asserts=False):
    """Build + compile the per-core Bass program (identical on all 8 cores).

    b1_zero: when True the score chain skips the bias add (2 DVE ops per
        group); otherwise the general batched DVE chain adds b1 first.
    """
    from concourse import bacc, mybir, tile

    f32 = mybir.dt.float32
    fp16 = mybir.dt.float16
    fp8 = mybir.dt.float8e4
    AF = mybir.ActivationFunctionType
    ALU = mybir.AluOpType
    AX = mybir.AxisListType

    nc = bacc.Bacc(
        "TRN2",
        target_bir_lowering=False,
        debug=False,
        enable_asserts=enable_asserts,
        num_devices=N_CORES,
    )
    gTr = nc.dram_tensor("gTr", [128, JC, ROWS], fp8, kind="ExternalInput").ap()
    hT = nc.dram_tensor("hT", [D, N], fp16, kind="ExternalInput").ap()
    hc = nc.dram_tensor("hc", [128, JC, D + 1], fp16, kind="ExternalInput").ap()
    W1s = nc.dram_tensor("W1s", [D, HID], fp16, kind="ExternalInput").ap()
    w2bt = nc.dram_tensor("w2bt", [128, GRP, HID], fp16, kind="ExternalInput").ap()
    front = nc.dram_tensor(
        "front", [128, 1024 + GRP * (D + 1)], fp16, kind="ExternalInput"
    ).ap()
    if not b1_zero:
        b1bt = nc.dram_tensor(
            "b1bt", [128, GRP, HID], fp16, kind="ExternalInput"
        ).ap()
        ebias = nc.dram_tensor("ebias", [128, 1], f32, kind="ExternalInput").ap()
    out = nc.dram_tensor("out", [128, 8, D], fp16, kind="ExternalOutput").ap()

    with tile.TileContext(nc) as tc:
        with (
            tc.tile_pool(name="const", bufs=1) as cpool,
            tc.tile_pool(name="gbuf", bufs=8) as gpool,
            tc.tile_pool(name="hpbuf", bufs=16) as hpool,
            tc.tile_pool(name="sbuf", bufs=2) as spool,
            tc.tile_pool(name="outbuf", bufs=1) as opool,
            tc.tile_pool(name="ps_z", bufs=2, space="PSUM") as zpool,
            tc.tile_pool(name="ps_w", bufs=1, space="PSUM") as wpool,
            tc.tile_pool(name="ps_acc", bufs=3, space="PSUM") as ps_acc,
        ):
            # ---- smalls on the gpsimd queue ----
            W1s_sb = cpool.tile([D, HID], fp16)
            nc.gpsimd.dma_start(W1s_sb[:], W1s[:])
            w2bt_sb = cpool.tile([128, GRP, HID], fp16)
            nc.gpsimd.dma_start(w2bt_sb[:], w2bt[:])
            if not b1_zero:
                b1bt_sb = cpool.tile([128, GRP, HID], fp16)
                nc.gpsimd.dma_start(b1bt_sb[:], b1bt[:])
                ebias_sb = cpool.tile([128, 1], f32)
                nc.gpsimd.dma_start(ebias_sb[:], ebias[:])

            # ---- heavy traffic: ONE queue (sync), exact need-order ----
            hT_sb = cpool.tile([D, N], fp16)
            hc_sb = cpool.tile([128, JC, D + 1], fp16)
            gts = {}

            def dma_group(g):
                sl = slice(g * GRP, (g + 1) * GRP)
                hsl = slice(g * (N // 8), (g + 1) * (N // 8))
                nc.sync.dma_start(hT_sb[:, hsl], hT[:, hsl])
                gt = gpool.tile([128, GRP, ROWS], fp8, tag="gt", name=f"gt{g}")
                nc.sync.dma_start(gt[:], gTr[:, sl, :])
                nc.sync.dma_start(hc_sb[:, sl, :], hc[:, sl, :])
                gts[g] = gt

            # group 0 split in half-groups at every level (DMA, z MMs, main
            # MMs) so the first 32 main matmuls start on ~0.7 MB of data.
            # All heavy traffic stays on sync -- the fastest pipe. Early
            # triggers cost ~1 us each on the queue head, so group 0's
            # hT+hc arrive as ONE host-packed blob (one trigger).
            front_sb = cpool.tile([128, 1024 + GRP * (D + 1)], fp16)
            nc.sync.dma_start(front_sb[:], front[:])
            g0ts = []
            for h in range(2):
                qsl = slice(h * 4, (h + 1) * 4)
                g0t = gpool.tile([128, 4, ROWS], fp8, tag="g0t", name=f"g0t{h}",
                                 bufs=2)
                nc.sync.dma_start(g0t[:], gTr[:, qsl, :])
                g0ts.append(g0t)
            for g in range(1, NG):
                dma_group(g)

            # preset the constant ones column (col 129) in every hp ring
            # buffer once; later ts_mul builds write only cols 0:129
            for b in range(16):
                t = hpool.tile([128, NCOL], fp16, tag="hp", name=f"hpinit{b}")
                nc.vector.memset(t[:, 129:130], 1.0)

            # wones[:, 0, jc] = w_jc (written by ACT exp). Writes stay
            # contiguous in the innermost dim (non-contiguous engine writes
            # mis-lower on HW); reads may stride.
            wones = cpool.tile([128, 2, JC], f32)
            zero1 = cpool.tile([128, 1], f32)
            nc.vector.memset(zero1[:], 0.0)

            # ---- PE warmup (HAM un-throttle) + activation-table preloads,
            # all dependency-free so they run during the first DMAs ----
            warm = cpool.tile([128, 128], fp16)
            nc.vector.memset(warm[:], 0.0)
            junk0 = spool.tile([128, 1], f32, tag="junk0", bufs=1)
            nc.scalar.activation(junk0[:], zero1[:], AF.Exp)
            nc.scalar.activation(junk0[:], zero1[:], AF.Copy)
            pwarm = wpool.tile([128, 128], f32, tag="pwarm")
            for _ in range(22):
                nc.tensor.matmul(pwarm[:], warm[:], warm[:], start=True, stop=True)

            # pack 3 accumulators per PSUM bank (3 * 130 f32 = 1560 B of 2 KB)
            acctiles = [
                ps_acc.tile([128, 3, NCOL], f32, tag="acc", name=f"accb{i}")
                for i in range(3)
            ]
            accs = [acctiles[i // 3][:, i % 3, :] for i in range(8)]

            def emit_z0_half(hh):
                """Group-0 half-block: 4 z MMs + per-chunk chains, hp on ACT."""
                zps = zpool.tile([128, 4, HID], f32, tag="z0", name=f"z0h{hh}",
                                 bufs=2)
                for k4 in range(4):
                    c = hh * 4 + k4
                    nc.tensor.matmul(
                        zps[:, k4, :],
                        front_sb[:, c * 128 : (c + 1) * 128],
                        W1s_sb[:],
                        start=True,
                        stop=True,
                    )
                for k4 in range(4):
                    k = hh * 4 + k4
                    prodk = spool.tile([128, HID], fp16, tag="prodk",
                                       name=f"prodk{k}", bufs=3)
                    nc.vector.scalar_tensor_tensor(
                        prodk[:], zps[:, k4, :], 0.0, w2bt_sb[:, k % GRP, :],
                        op0=ALU.max, op1=ALU.mult,
                    )
                    e1 = spool.tile([128, 1], f32, tag="e1",
                                    name=f"e1{k}", bufs=3)
                    nc.vector.tensor_reduce(
                        e1[:], prodk[:], axis=AX.X, op=ALU.add
                    )
                    # shift-free: any constant in e cancels in the ratio
                    nc.scalar.activation(
                        wones[:, 0, k : k + 1], e1[:], AF.Exp
                    )
                    hps0.append(build_hp(
                        k, on_act=True,
                        src_ap=front_sb[
                            :, 1024 + k * (D + 1) : 1024 + (k + 1) * (D + 1)
                        ],
                    ))

            def emit_z(g):
                """MLP z for the 8 chunks of group g: z[j, k] on j-partitions,
                then a batched score chain -> w."""
                zps = zpool.tile([128, GRP, HID], f32, tag="z", name=f"z{g}")
                for k in range(GRP):
                    c = g * GRP + k
                    nc.tensor.matmul(
                        zps[:, k, :],
                        hT_sb[:, c * 128 : (c + 1) * 128],
                        W1s_sb[:],
                        start=True,
                        stop=True,
                    )
                if b1_zero:
                    prod = spool.tile([128, GRP, HID], fp16, tag="prod")
                    nc.vector.scalar_tensor_tensor(
                        prod[:], zps[:], 0.0, w2bt_sb[:], op0=ALU.max,
                        op1=ALU.mult,
                    )
                    e8 = spool.tile([128, GRP], f32, tag="e8")
                    nc.vector.tensor_reduce(e8[:], prod[:], axis=AX.X, op=ALU.add)
                    nc.scalar.activation(
                        wones[:, 0, g * GRP : (g + 1) * GRP], e8[:], AF.Exp
                    )
                else:
                    # general-b1 fallback: batched DVE relu-dot chain
                    zb = spool.tile([128, GRP, HID], fp16, tag="zb")
                    nc.vector.tensor_tensor(zb[:], zps[:], b1bt_sb[:], op=ALU.add)
                    prod = spool.tile([128, GRP, HID], fp16, tag="prod")
                    nc.vector.scalar_tensor_tensor(
                        prod[:], zb[:], 0.0, w2bt_sb[:], op0=ALU.max, op1=ALU.mult
                    )
                    e8 = spool.tile([128, GRP], f32, tag="e8")
                    nc.vector.tensor_reduce(e8[:], prod[:], axis=AX.X, op=ALU.add)
                    nc.scalar.activation(
                        wones[:, 0, g * GRP : (g + 1) * GRP], e8[:], AF.Exp,
                        bias=ebias_sb[:],
                    )

            def build_hp(jc, on_act=False, src_ap=None):
                """Just-in-time H' chunk: [w*h | w | 1] fp16.

                hc carries a host-side ones column, so ONE scaled copy makes
                [w*h | w]; the constant 1 in col 129 was preset per buffer.
                Builds alternate between Vector (ts_mul) and Scalar (Copy
                with per-partition scale) to balance engine load.
                """
                hp = hpool.tile([128, NCOL], fp16, tag="hp", name=f"hp{jc}")
                src = hc_sb[:, jc, :] if src_ap is None else src_ap
                if on_act:
                    nc.scalar.activation(
                        hp[:, 0:129], src, AF.Copy,
                        scale=wones[:, 0, jc : jc + 1],
                    )
                else:
                    nc.vector.tensor_scalar_mul(
                        hp[:, 0:129], src, wones[:, 0, jc : jc + 1]
                    )
                return hp

            def epilogue(it):
                """out rows of bank it: deg/den rescale into ot_all."""
                den = spool.tile([128, 1], f32, tag="den", name=f"den{it}", bufs=8)
                nc.vector.tensor_scalar_add(den[:], accs[it][:, 128:129], 1e-30)
                rc = spool.tile([128, 1], f32, tag="rc", name=f"rc{it}", bufs=8)
                nc.vector.reciprocal(rc[:], den[:])
                r = spool.tile([128, 1], f32, tag="r", name=f"r{it}", bufs=8)
                nc.vector.tensor_tensor(r[:], rc[:], accs[it][:, 129:130], op=ALU.mult)
                nc.vector.tensor_scalar_mul(
                    ot_all[:, it, :], accs[it][:, 0:128], r[:]
                )

            ot_all = opool.tile([128, 8, D], fp16, tag="ot_all", bufs=1)

            hps0 = []
            # group 0: half-blocks interleaved z -> main
            for hh in range(2):
                emit_z0_half(hh)
                for k4 in range(4):
                    jc = hh * 4 + k4
                    hp = hps0[jc]
                    for it in range(8):
                        # start=True clears has_written for the WHOLE psum
                        # bank, so only the first slice sharing each bank
                        # may issue it; siblings then init via overwrite
                        # (has_written=0) on their first matmul.
                        nc.tensor.matmul(
                            accs[it][:],
                            g0ts[hh][:, k4, it * 128 : (it + 1) * 128],
                            hp[:],
                            start=(jc == 0 and it % 3 == 0),
                            stop=False,
                        )
            for g in range(1, NG):
                if g == 1:
                    emit_z(1)
                    emit_z(2)
                elif g + 1 < NG:
                    emit_z(g + 1)
                gt = gts.pop(g)

                if g < NG - 1:
                    for k in range(GRP):
                        jc = g * GRP + k
                        hp = build_hp(jc, on_act=(k % 2 == 1))
                        for it in range(8):
                            nc.tensor.matmul(
                                accs[it][:],
                                gt[:, k, it * 128 : (it + 1) * 128],
                                hp[:],
                                start=False,
                                stop=False,
                            )
                else:
                    # last group bank-major: each bank's epilogue overlaps the
                    # remaining banks' matmuls. Visit order hops across PSUM
                    # banks (accs share banks in triples) so an epilogue's
                    # DVE reads never WAR-block the next bank's matmuls.
                    hps = [build_hp(g * GRP + k, on_act=(k % 2 == 1))
                           for k in range(GRP)]
                    done = [0, 0, 0]
                    for it in (0, 3, 6, 1, 4, 7, 2, 5):
                        for k in range(GRP):
                            nc.tensor.matmul(
                                accs[it][:],
                                gt[:, k, it * 128 : (it + 1) * 128],
                                hps[k][:],
                                start=False,
                                stop=(k == GRP - 1),
                            )
                        epilogue(it)
                        b = it // 3
                        done[b] += 1
                        if done[b] == (2 if b == 2 else 3):
                            sl = slice(3 * b, min(3 * b + 3, 8))
                            nc.scalar.dma_start(out[:, sl, :], ot_all[:, sl, :])

    nc.compile()
    return nc


def make_in_maps(graph_info, h, W1, b1, W2, b2):
    """Shard + lay out the full inputs for the 8 cores."""
    import ml_dtypes

    fp16 = np.float16
    fp8 = ml_dtypes.float8_e4m3

    W1 = np.asarray(W1, np.float32)
    W2 = np.asarray(W2, np.float32).reshape(HID)
    b1 = np.asarray(b1, np.float32)
    b1_zero = not np.any(b1)

    h = np.asarray(h, np.float32)
    hT = np.ascontiguousarray(h.T).astype(fp16)                # [D, N]
    hcm = np.ascontiguousarray(
        np.concatenate(
            [h.reshape(JC, 128, D), np.ones((JC, 128, 1), np.float32)], axis=2
        ).transpose(1, 0, 2)                                   # [128, JC, D+1]
    ).astype(fp16)

    front = np.ascontiguousarray(
        np.concatenate([hT[:, 0:1024], hcm[:, 0:GRP, :].reshape(128, -1)], axis=1)
    )
    base = {
        "hT": hT,
        "hc": hcm,
        "front": front,
        "W1s": W1.astype(fp16),
        "w2bt": np.ascontiguousarray(
            np.broadcast_to(W2, (128, GRP, HID))
        ).astype(fp16),
    }
    if not b1_zero:
        base["b1bt"] = np.ascontiguousarray(
            np.broadcast_to(b1, (128, GRP, HID))
        ).astype(fp16)
        base["ebias"] = np.full(
            (128, 1), float(np.asarray(b2).reshape(-1)[0]) + ESHIFT, np.float32
        )

    g8 = np.asarray(graph_info, np.float32).astype(fp8)        # exact 0/1
    in_maps = []
    for c in range(N_CORES):
        shard = g8[c * ROWS : (c + 1) * ROWS]                  # [1024, N]
        gTr = np.ascontiguousarray(
            shard.reshape(ROWS, JC, 128).transpose(2, 1, 0)    # [128, JC, 1024]
        )
        in_maps.append(dict(base, gTr=gTr))
    return in_maps


def kernel(graph_info, h, W1, b1, W2, b2):
    _install_axon_hooks_shim()
    from concourse.bass_utils import run_bass_kernel_spmd

    b1_zero = not np.any(np.asarray(b1))
    key = ("nc", b1_zero)
    if key not in _cache:
        _cache[key] = build_nc(b1_zero)
        _cache["nc"] = _cache[key]
    nc = _cache[key]

    in_maps = make_in_maps(graph_info, h, W1, b1, W2, b2)
    res = run_bass_kernel_spmd(nc, in_maps, list(range(N_CORES)))
    return np.concatenate(
        [
            res.results[c]["out"].transpose(1, 0, 2).reshape(ROWS, D)
            for c in range(N_CORES)
        ],
        axis=0,
    ).astype(np.float32)
